# revision 30
# baseline (speedup 1.0000x reference)
"""HRR self-attention (causal holographic binding) on 8 Trainium2 cores.

Math (per batch b, head h, reference semantics):
    qkv = x @ w_qkv ; q,k,v heads of HD=128
    fq,fk,fv = fft(q|k|v, axis=-1)          (length-128 FFT == matmul with DFT matrix)
    kv   = cumsum(fk*fv, axis=seq)          (causal binding)
    vals = ifft(kv * conj(fq)).real
    out  = vals @ w_out

v13 pipeline (per head h, token chunk t of 512):
  * Packed real spectrum on 128 partition rows: p=0..63 Re bins 0..63,
    p=64 Nyquist (Re bin 64), p=65..127 Im bins 1..63.
  * The k/q DFTs are folded into the projection weights on the host
    (Wk' = Wk @ Gk/16, Wq' = Wq @ Gk/16), so fk and fq come straight out
    of the x-projection; only the v path needs on-device DFTs (vs feeds
    two spectra):  fv = Gv.T vs,  fvs = Gsm.T vs,
    Gsm = [+sin | nyq | cos] -> fvs = [-Im(V) | Nyq(V) | Re(V)].
  * DVE ops may read/write different partition bases (only the two INPUTS
    of a 2-input op must share a base), so the binding products write
    their halves partition-shifted, with no intermediate swap:
        in0 = [ fk.Re*fv.Re ; fk.Im*fvs.hi ]   (A ; D)
        in1 = [ fk.Im*fv.Im ; fk.Re*fvs.lo ]   (B ; -E)  shifted outputs
    kvt = tensor_tensor_scan(in0, in1, op0=add, op1=subtract)
        -> rows 0:64  cumsum(A - B)  = Re(KV) bins (row0 = DC)
           rows 64:   cumsum(D + E)  = Im(KV) bins (row64 = Nyquist)
    One 128-row scan per head-chunk.  fqs = partition-shifted DVE copies
    of fq (its polluted rows 0/64 die against A2's zero rows).
  * Unit order vs, fk, v-DFTs, fq — the chain-critical fv/fvs/fk
    evacuations aren't queued behind the late-needed fq; ifft matmuls
    are emitted one unit late so their p12 chain has PE cover.
  * Comp-granular PSUM (1 bank per tile, pools psP2/psS2/psV4); PSUM
    evacuations on Scalar, ot + flushed vals on DVE; startup DMA issues
    spread over the sync/scalar/gpsimd queues (~0.6us per dma_start).
  * Sharding: core c = 2*b + g handles batch b, heads 4g..4g+3; host sums
    the pair of partial outT per batch.  fp16 matmuls, fp32 PSUM; DFT
    matrices pre-scaled by 1/16 per application (undone on host).
"""

import numpy as np

B, S, D, H = 4, 4096, 1024, 8
HD = 128
NCORES = 8
HPC = H // 2            # heads per core
T = 512                 # token chunk (PSUM bank = 512 fp32)
NT = S // T
KK = D // 128           # contraction tiles for the qkv projection
FS = 16.0               # scale folded into each forward DFT matrix
SV = 16.0               # vals stored as vals/SV
SO = 16.0               # outT stored as out/SO  (host multiplies back)


def _build_consts():
    """Returns (gmat [Gv|Gsm]/FS, amat [A1|A2]*Amul, Gk/FS for host fusion)."""
    n = HD
    a = np.arange(n)
    cos_aj = np.cos(2 * np.pi * np.outer(a, np.arange(64)) / n)   # [a, j]
    sin_aj = np.sin(2 * np.pi * np.outer(a, np.arange(64)) / n)
    nyq = np.where(a % 2 == 0, 1.0, -1.0)              # (-1)^a

    def fwd(re_cols, col64, im_cols):
        M = np.zeros((n, n))
        M[:, :64] = re_cols
        M[:, 64] = col64
        M[:, 65:] = im_cols[:, 1:]                     # im bins 1..63
        return M

    Gk = fwd(cos_aj, nyq, -sin_aj)                     # folded into Wk', Wq'
    Gv = fwd(cos_aj, 0.0, -sin_aj)
    Gsm = fwd(+sin_aj, nyq, cos_aj)                    # v-swap, negated lo half

    # inverse: vals_n = sum_p A1[p,n] P1[p] + A2[p,n] P2[p]
    cos_jn = np.cos(2 * np.pi * np.outer(np.arange(64), a) / n)   # [j, n]
    sin_jn = np.sin(2 * np.pi * np.outer(np.arange(64), a) / n)
    w = np.full(64, 2.0)
    w[0] = 1.0
    A1 = np.zeros((n, n))
    A1[:64, :] = w[:, None] * cos_jn / n
    A1[64, :] = np.where(np.arange(n) % 2 == 0, 1.0, -1.0) / n    # Nyquist (-1)^n
    A1[65:, :] = 2.0 * cos_jn[1:] / n
    A2 = np.zeros((n, n))
    A2[:64, :] = 2.0 * sin_jn / n
    A2[64, :] = 0.0
    A2[65:, :] = -2.0 * sin_jn[1:] / n

    Amul = FS ** 3 / SV
    gmat = np.concatenate([Gv / FS, Gsm / FS], axis=1).astype(np.float16)  # [128, 256]
    amat = np.concatenate([A1 * Amul, A2 * Amul], axis=1).astype(np.float16)  # [128, 256]
    return gmat, amat, Gk / FS


def _build_program():
    import concourse.bass as bass
    import concourse.bacc as bacc
    import concourse.mybir as mybir
    import concourse.tile as tile

    f16 = mybir.dt.float16
    f32 = mybir.dt.float32
    add = mybir.AluOpType.add
    sub = mybir.AluOpType.subtract

    nc = bacc.Bacc("TRN2", target_bir_lowering=False, debug=False)
    xT = nc.dram_tensor("xT", [D, S], f16, kind="ExternalInput").ap()
    wq = nc.dram_tensor("wq", [D, 3 * HPC * 128], f16, kind="ExternalInput").ap()
    wo = nc.dram_tensor("wo", [HPC * 128, D], f16, kind="ExternalInput").ap()
    gmat = nc.dram_tensor("gmat", [128, 256], f16, kind="ExternalInput").ap()
    amat = nc.dram_tensor("amat", [128, 256], f16, kind="ExternalInput").ap()
    outT = nc.dram_tensor("outT", [D, S], f16, kind="ExternalOutput").ap()

    with tile.TileContext(nc) as tc:
        with (
            tc.tile_pool(name="consts", bufs=1) as cpool,
            tc.tile_pool(name="xin", bufs=3) as xpool,
            tc.tile_pool(name="work", bufs=2) as wpool,
            tc.tile_pool(name="kvp", bufs=2) as kvpool,
            tc.tile_pool(name="psP", bufs=2, space="PSUM") as psP,
            tc.tile_pool(name="psS", bufs=2, space="PSUM") as psS,
            tc.tile_pool(name="psV", bufs=4, space="PSUM") as psV,
        ):
            # startup-critical DMAs first: head-0 weights, then chunk-0 x in
            # half tiles (two DMA engines per k-tile), then the other heads.
            wq_sb = [cpool.tile([128, 3 * HPC * 128], f16, name=f"wq{k}")
                     for k in range(KK)]
            for k in range(KK):
                nc.sync.dma_start(out=wq_sb[k][:, 0:384],
                                  in_=wq[k * 128:(k + 1) * 128, 0:384])
            xk0 = []
            HT = T // 2
            qs_cycle = [nc.scalar, nc.gpsimd, nc.sync]
            for k in range(KK):
                xkt = xpool.tile([128, T], f16, tag=f"xk{k}", name=f"x_0_{k}")
                qs_cycle[k % 3].dma_start(out=xkt[:, 0:HT],
                                          in_=xT[k * 128:(k + 1) * 128, 0:HT])
                qs_cycle[(k + 1) % 3].dma_start(out=xkt[:, HT:T],
                                                in_=xT[k * 128:(k + 1) * 128, HT:T])
                xk0.append(xkt)
            for h in range(1, HPC):
                for k in range(KK):
                    c0 = h * 384
                    qs_cycle[(h + k) % 3].dma_start(
                        out=wq_sb[k][:, c0:c0 + 384],
                        in_=wq[k * 128:(k + 1) * 128, c0:c0 + 384])
            g_sb = cpool.tile([128, 256], f16, name="g_sb")
            nc.sync.dma_start(out=g_sb, in_=gmat)
            a_sb = cpool.tile([128, 256], f16, name="a_sb")
            nc.sync.dma_start(out=a_sb, in_=amat)
            wo_sb = []
            for h in range(HPC):
                wot = cpool.tile([128, D], f16, name=f"wo{h}")
                nc.gpsimd.dma_start(out=wot, in_=wo[h * 128:(h + 1) * 128, :])
                wo_sb.append(wot)

            kv_prev = [None] * HPC
            p12_pend = []         # [(t, h, p12, vals_list)] awaiting ifft

            def emit_ifft(tt, hh, p12_o, vlist, dve_evac=False):
                ps_vals = psV.tile([128, T], f32, tag="v", name=f"psv_{tt}_{hh}")
                nc.tensor.matmul(ps_vals, lhsT=a_sb[:, 0:128], rhs=p12_o[:, 0:T],
                                 start=True, stop=False)
                nc.tensor.matmul(ps_vals, lhsT=a_sb[:, 128:256],
                                 rhs=p12_o[:, T:2 * T], start=False, stop=True)
                vt = wpool.tile([128, T], f16, tag=f"vals{hh}", name=f"vals_{tt}_{hh}")
                if dve_evac:
                    nc.vector.tensor_copy(vt, ps_vals)
                else:
                    nc.scalar.copy(vt, ps_vals)
                vlist.append(vt)

            for t in range(NT):
                ts = slice(t * T, (t + 1) * T)
                if t == 0:
                    xk = xk0
                else:
                    xk = []
                    for k in range(KK):
                        xkt = xpool.tile([128, T], f16, tag=f"xk{k}", name=f"x_{t}_{k}")
                        nc.sync.dma_start(out=xkt, in_=xT[k * 128:(k + 1) * 128, ts])
                        xk.append(xkt)
                vals_sb = []
                for h in range(HPC):
                    # x-projections: comp0 -> fq (Wq'), comp1 -> fk (Wk'),
                    # comp2 -> vs (Wv).  Order: vs, fk, then the v-DFTs, then
                    # fq last — so the chain-critical fv/fvs/fk evacuations
                    # aren't queued behind the (late-needed) fq evacuation.
                    def proj(nm, comp):
                        ps_c = psP.tile([128, T], f32, tag="p", name=f"psp_{t}_{h}_{nm}")
                        col0 = (h * 3 + comp) * 128
                        for k in range(KK):
                            nc.tensor.matmul(
                                ps_c,
                                lhsT=wq_sb[k][:, col0:col0 + 128],
                                rhs=xk[k],
                                start=(k == 0),
                                stop=(k == KK - 1),
                            )
                        csb = wpool.tile([128, T], f16, tag=f"c{h}_{nm}",
                                         name=f"{nm}_{t}_{h}")
                        nc.scalar.copy(csb, ps_c)
                        return csb
                    vs = proj("vs", 2)
                    fk_s = proj("fk", 1)
                    # v-path DFTs: fv (Gv), fvs (Gsm)
                    spec = {}
                    for nm, gcol in (("fv", 0), ("fvs", 128)):
                        ps_f = psS.tile([128, T], f32, tag="s", name=f"psf_{nm}_{t}_{h}")
                        nc.tensor.matmul(ps_f, lhsT=g_sb[:, gcol:gcol + 128], rhs=vs)
                        ssb = wpool.tile([128, T], f16, tag=f"{nm}{h}",
                                         name=f"{nm}_{t}_{h}")
                        nc.scalar.copy(ssb, ps_f)
                        spec[nm] = ssb
                    fv_s, fvs_s = spec["fv"], spec["fvs"]
                    fq_s = proj("fq", 0)
                    # lagged ifft: unit u-1's p12 gets an extra proj group of
                    # PE cover before its ifft matmuls hit the queue
                    if len(p12_pend) >= 1:
                        tt, hh, p12_o, vlist = p12_pend.pop(0)
                        emit_ifft(tt, hh, p12_o, vlist)
                    # fqs = half-swap of fq (rows j <-> 64+j) via GpSimd
                    # partition-shifted copies; rows 0/64 are polluted but
                    # multiplied by A2's zero rows downstream.
                    fqs = wpool.tile([128, T], f16, tag=f"fqs{h}", name=f"fqs_{t}_{h}")
                    nc.vector.tensor_copy(fqs[0:64, :], fq_s[64:128, :])
                    nc.vector.tensor_copy(fqs[64:128, :], fq_s[0:64, :])
                    # binding products written with partition-shifted outputs:
                    # in0 = [A ; D], in1 = [B ; -E] (no intermediate swap)
                    in0 = wpool.tile([128, T], f16, tag=f"in0_{h}", name=f"in0_{t}_{h}")
                    nc.vector.tensor_mul(in0[0:64, :], fk_s[0:64, :], fv_s[0:64, :])
                    nc.vector.tensor_mul(in0[64:128, :], fk_s[64:128, :], fvs_s[64:128, :])
                    in1 = wpool.tile([128, T], f16, tag=f"in1_{h}", name=f"in1_{t}_{h}")
                    nc.vector.tensor_mul(in1[0:64, :], fk_s[64:128, :], fv_s[64:128, :])
                    nc.vector.tensor_mul(in1[64:128, :], fk_s[0:64, :], fvs_s[0:64, :])
                    # causal cumsum: state = (in0 + state) - in1, carry-chained
                    kvt = kvpool.tile([128, T], f16, tag=f"kv{h}", name=f"kv_{t}_{h}")
                    init = 0.0 if t == 0 else kv_prev[h][:, T - 1:T]
                    nc.vector.tensor_tensor_scan(kvt, in0, in1, init, add, sub)
                    kv_prev[h] = kvt
                    # unbinding products
                    p12 = wpool.tile([128, 2 * T], f16, tag=f"p12{h}", name=f"p12_{t}_{h}")
                    nc.vector.tensor_mul(p12[:, 0:T], kvt, fq_s)
                    nc.vector.tensor_mul(p12[:, T:2 * T], kvt, fqs)
                    p12_pend.append((t, h, p12, vals_sb))
                # flush this chunk's remaining iffts (DVE evac so the Scalar
                # queue isn't the gate on the psV slot), then project out
                while p12_pend:
                    tt, hh, p12_o, vlist = p12_pend.pop(0)
                    emit_ifft(tt, hh, p12_o, vlist, dve_evac=True)
                # output projection (partial over this core's heads)
                last = t == NT - 1
                for od in range(D // 128):
                    ps_out = psV.tile([128, T], f32, tag="v", name=f"pso_{t}_{od}")
                    for h in range(HPC):
                        nc.tensor.matmul(ps_out,
                                         lhsT=wo_sb[h][:, od * 128:(od + 1) * 128],
                                         rhs=vals_sb[h],
                                         start=(h == 0), stop=(h == HPC - 1))
                    ot = wpool.tile([128, T], f16, tag="ot", name=f"ot_{t}_{od}")
                    if last and od % 2 == 1:
                        nc.scalar.copy(ot, ps_out)
                    else:
                        nc.vector.tensor_copy(ot, ps_out)
                    nc.sync.dma_start(out=outT[od * 128:(od + 1) * 128, ts], in_=ot)
    nc.compile()
    return nc


def _make_in_maps(x, w_qkv, w_out):
    gmat, amat, Gk16 = _build_consts()
    x16 = x.astype(np.float16)
    wqf = np.asarray(w_qkv, np.float64)
    wo16 = (w_out * (SV / SO)).astype(np.float16)
    in_maps = []
    for c in range(NCORES):
        b, g = divmod(c, 2)
        heads = range(4 * g, 4 * g + 4)
        blocks = []
        for h in heads:
            Wq_h = wqf[:, 0 * D + h * 128:0 * D + (h + 1) * 128]
            Wk_h = wqf[:, 1 * D + h * 128:1 * D + (h + 1) * 128]
            Wv_h = wqf[:, 2 * D + h * 128:2 * D + (h + 1) * 128]
            blocks.append((Wq_h @ Gk16).astype(np.float16))
            blocks.append((Wk_h @ Gk16).astype(np.float16))
            blocks.append(Wv_h.astype(np.float16))
        wq_cols = np.concatenate(blocks, axis=1)
        wo_rows = np.concatenate(
            [wo16[h * 128:(h + 1) * 128, :] for h in heads], axis=0)
        in_maps.append({
            "xT": np.ascontiguousarray(x16[b].T),
            "wq": np.ascontiguousarray(wq_cols),
            "wo": np.ascontiguousarray(wo_rows),
            "gmat": gmat,
            "amat": amat,
        })
    return in_maps


_NC_CACHE = None


def _get_program():
    global _NC_CACHE
    if _NC_CACHE is None:
        _NC_CACHE = _build_program()
    return _NC_CACHE


def kernel(x, w_qkv, w_out, _trace=False, _results_out=None):
    import sys
    if "/opt/trn_rl_repo" not in sys.path:
        sys.path.insert(0, "/opt/trn_rl_repo")
    from concourse.bass_utils import run_bass_kernel_spmd

    x = np.asarray(x)
    w_qkv = np.asarray(w_qkv)
    w_out = np.asarray(w_out)
    nc = _get_program()
    in_maps = _make_in_maps(x, w_qkv, w_out)
    res = run_bass_kernel_spmd(nc, in_maps, list(range(NCORES)), trace=_trace)
    if _results_out is not None:
        _results_out.append(res)
    out = np.empty((B, S, D), np.float32)
    for b in range(B):
        p0 = res.results[2 * b]["outT"].astype(np.float32)
        p1 = res.results[2 * b + 1]["outT"].astype(np.float32)
        out[b] = (p0 + p1).T * SO
    return out


# revision 34
# speedup vs baseline: 1.0084x; 1.0084x over previous
"""HRR self-attention (causal holographic binding) on 8 Trainium2 cores.

Math (per batch b, head h, reference semantics):
    qkv = x @ w_qkv ; q,k,v heads of HD=128
    fq,fk,fv = fft(q|k|v, axis=-1)          (length-128 FFT == matmul with DFT matrix)
    kv   = cumsum(fk*fv, axis=seq)          (causal binding)
    vals = ifft(kv * conj(fq)).real
    out  = vals @ w_out

v13 pipeline (per head h, token chunk t of 512):
  * Packed real spectrum on 128 partition rows: p=0..63 Re bins 0..63,
    p=64 Nyquist (Re bin 64), p=65..127 Im bins 1..63.
  * The k/q DFTs are folded into the projection weights on the host
    (Wk' = Wk @ Gk/16, Wq' = Wq @ Gk/16), so fk and fq come straight out
    of the x-projection; only the v path needs on-device DFTs (vs feeds
    two spectra):  fv = Gv.T vs,  fvs = Gsm.T vs,
    Gsm = [+sin | nyq | cos] -> fvs = [-Im(V) | Nyq(V) | Re(V)].
  * DVE ops may read/write different partition bases (only the two INPUTS
    of a 2-input op must share a base), so the binding products write
    their halves partition-shifted, with no intermediate swap:
        in0 = [ fk.Re*fv.Re ; fk.Im*fvs.hi ]   (A ; D)
        in1 = [ fk.Im*fv.Im ; fk.Re*fvs.lo ]   (B ; -E)  shifted outputs
    kvt = tensor_tensor_scan(in0, in1, op0=add, op1=subtract)
        -> rows 0:64  cumsum(A - B)  = Re(KV) bins (row0 = DC)
           rows 64:   cumsum(D + E)  = Im(KV) bins (row64 = Nyquist)
    One 128-row scan per head-chunk.  fqs = partition-shifted DVE copies
    of fq (its polluted rows 0/64 die against A2's zero rows).
  * Unit order vs, fk, v-DFTs, fq — the chain-critical fv/fvs/fk
    evacuations aren't queued behind the late-needed fq; ifft matmuls
    are emitted one unit late so their p12 chain has PE cover.
  * Comp-granular PSUM (1 bank per tile, pools psP2/psS2/psV4); PSUM
    evacuations on Scalar, ot + flushed vals on DVE; startup DMA issues
    spread over the sync/scalar/gpsimd queues (~0.6us per dma_start).
  * Sharding: core c = 2*b + g handles batch b, heads 4g..4g+3; host sums
    the pair of partial outT per batch.  fp16 matmuls, fp32 PSUM; DFT
    matrices pre-scaled by 1/16 per application (undone on host).
"""

import numpy as np

B, S, D, H = 4, 4096, 1024, 8
HD = 128
NCORES = 8
HPC = H // 2            # heads per core
T = 512                 # token chunk (PSUM bank = 512 fp32)
NT = S // T
KK = D // 128           # contraction tiles for the qkv projection
FS = 16.0               # scale folded into each forward DFT matrix
SV = 16.0               # vals stored as vals/SV
SO = 16.0               # outT stored as out/SO  (host multiplies back)


def _build_consts():
    """Returns (gmat [Gv|Gsm]/FS, amat [A1|A2]*Amul, Gk/FS for host fusion)."""
    n = HD
    a = np.arange(n)
    cos_aj = np.cos(2 * np.pi * np.outer(a, np.arange(64)) / n)   # [a, j]
    sin_aj = np.sin(2 * np.pi * np.outer(a, np.arange(64)) / n)
    nyq = np.where(a % 2 == 0, 1.0, -1.0)              # (-1)^a

    def fwd(re_cols, col64, im_cols):
        M = np.zeros((n, n))
        M[:, :64] = re_cols
        M[:, 64] = col64
        M[:, 65:] = im_cols[:, 1:]                     # im bins 1..63
        return M

    Gk = fwd(cos_aj, nyq, -sin_aj)                     # folded into Wk', Wq'
    Gv = fwd(cos_aj, 0.0, -sin_aj)
    Gsm = fwd(+sin_aj, nyq, cos_aj)                    # v-swap, negated lo half

    # inverse: vals_n = sum_p A1[p,n] P1[p] + A2[p,n] P2[p]
    cos_jn = np.cos(2 * np.pi * np.outer(np.arange(64), a) / n)   # [j, n]
    sin_jn = np.sin(2 * np.pi * np.outer(np.arange(64), a) / n)
    w = np.full(64, 2.0)
    w[0] = 1.0
    A1 = np.zeros((n, n))
    A1[:64, :] = w[:, None] * cos_jn / n
    A1[64, :] = np.where(np.arange(n) % 2 == 0, 1.0, -1.0) / n    # Nyquist (-1)^n
    A1[65:, :] = 2.0 * cos_jn[1:] / n
    A2 = np.zeros((n, n))
    A2[:64, :] = 2.0 * sin_jn / n
    A2[64, :] = 0.0
    A2[65:, :] = -2.0 * sin_jn[1:] / n

    Amul = FS ** 3 / SV
    gmat = np.concatenate([Gv / FS, Gsm / FS], axis=1).astype(np.float16)  # [128, 256]
    amat = np.concatenate([A1 * Amul, A2 * Amul], axis=1).astype(np.float16)  # [128, 256]
    return gmat, amat, Gk / FS


def _build_program():
    import concourse.bass as bass
    import concourse.bacc as bacc
    import concourse.mybir as mybir
    import concourse.tile as tile

    f16 = mybir.dt.float16
    f32 = mybir.dt.float32
    add = mybir.AluOpType.add
    sub = mybir.AluOpType.subtract

    nc = bacc.Bacc("TRN2", target_bir_lowering=False, debug=False)
    xT = nc.dram_tensor("xT", [D, S], f16, kind="ExternalInput").ap()
    wq = nc.dram_tensor("wq", [D, 3 * HPC * 128], f16, kind="ExternalInput").ap()
    wo = nc.dram_tensor("wo", [HPC * 128, D], f16, kind="ExternalInput").ap()
    gmat = nc.dram_tensor("gmat", [128, 256], f16, kind="ExternalInput").ap()
    amat = nc.dram_tensor("amat", [128, 256], f16, kind="ExternalInput").ap()
    outT = nc.dram_tensor("outT", [D, S], f16, kind="ExternalOutput").ap()

    with tile.TileContext(nc) as tc:
        with (
            tc.tile_pool(name="consts", bufs=1) as cpool,
            tc.tile_pool(name="xin", bufs=3) as xpool,
            tc.tile_pool(name="work", bufs=2) as wpool,
            tc.tile_pool(name="kvp", bufs=2) as kvpool,
            tc.tile_pool(name="psP", bufs=2, space="PSUM") as psP,
            tc.tile_pool(name="psS", bufs=2, space="PSUM") as psS,
            tc.tile_pool(name="psV", bufs=4, space="PSUM") as psV,
        ):
            # PE warm-up first: junk matmuls sized to end just before the
            # first real matmul (~10us in), so the HAM clock gate is at 8/8
            # and stays there (an early-finishing stream re-throttles: the
            # MID window is ~3.4us).
            warm_sb = cpool.tile([128, T], f16, name="warm_sb")
            nc.gpsimd.memset(warm_sb, 0)
            warm_ps = psS.tile([128, T], f32, tag="s", name="warm_ps")
            for _ in range(30):
                nc.tensor.matmul(warm_ps, lhsT=warm_sb[:, 0:128],
                                 rhs=warm_sb, start=True, stop=True)
            # startup-critical DMAs: head-0 weights, then chunk-0 x in
            # half tiles (two DMA engines per k-tile), then the other heads.
            wq_sb = [cpool.tile([128, 3 * HPC * 128], f16, name=f"wq{k}")
                     for k in range(KK)]
            for k in range(KK):
                nc.sync.dma_start(out=wq_sb[k][:, 0:384],
                                  in_=wq[k * 128:(k + 1) * 128, 0:384])
            xk0 = []
            HT = T // 2
            qs_cycle = [nc.scalar, nc.gpsimd, nc.sync]
            for k in range(KK):
                xkt = xpool.tile([128, T], f16, tag=f"xk{k}", name=f"x_0_{k}")
                qs_cycle[k % 3].dma_start(out=xkt[:, 0:HT],
                                          in_=xT[k * 128:(k + 1) * 128, 0:HT])
                qs_cycle[(k + 1) % 3].dma_start(out=xkt[:, HT:T],
                                                in_=xT[k * 128:(k + 1) * 128, HT:T])
                xk0.append(xkt)
            for h in range(1, HPC):
                for k in range(KK):
                    c0 = h * 384
                    qs_cycle[(h + k) % 3].dma_start(
                        out=wq_sb[k][:, c0:c0 + 384],
                        in_=wq[k * 128:(k + 1) * 128, c0:c0 + 384])
            g_sb = cpool.tile([128, 256], f16, name="g_sb")
            nc.sync.dma_start(out=g_sb, in_=gmat)
            a_sb = cpool.tile([128, 256], f16, name="a_sb")
            nc.sync.dma_start(out=a_sb, in_=amat)
            wo_sb = []
            for h in range(HPC):
                wot = cpool.tile([128, D], f16, name=f"wo{h}")
                nc.gpsimd.dma_start(out=wot, in_=wo[h * 128:(h + 1) * 128, :])
                wo_sb.append(wot)

            kv_prev = [None] * HPC
            p12_pend = []         # [(t, h, p12, vals_list)] awaiting ifft

            def emit_ifft(tt, hh, p12_o, vlist, dve_evac=False):
                ps_vals = psV.tile([128, T], f32, tag="v", name=f"psv_{tt}_{hh}")
                nc.tensor.matmul(ps_vals, lhsT=a_sb[:, 0:128], rhs=p12_o[:, 0:T],
                                 start=True, stop=False)
                nc.tensor.matmul(ps_vals, lhsT=a_sb[:, 128:256],
                                 rhs=p12_o[:, T:2 * T], start=False, stop=True)
                vt = wpool.tile([128, T], f16, tag=f"vals{hh}", name=f"vals_{tt}_{hh}")
                if dve_evac:
                    nc.vector.tensor_copy(vt, ps_vals)
                else:
                    nc.scalar.copy(vt, ps_vals)
                vlist.append(vt)

            for t in range(NT):
                ts = slice(t * T, (t + 1) * T)
                if t == 0:
                    xk = xk0
                else:
                    xk = []
                    for k in range(KK):
                        xkt = xpool.tile([128, T], f16, tag=f"xk{k}", name=f"x_{t}_{k}")
                        nc.sync.dma_start(out=xkt, in_=xT[k * 128:(k + 1) * 128, ts])
                        xk.append(xkt)
                vals_sb = []
                for h in range(HPC):
                    # x-projections: comp0 -> fq (Wq'), comp1 -> fk (Wk'),
                    # comp2 -> vs (Wv).  Order: vs, fk, then the v-DFTs, then
                    # fq last — so the chain-critical fv/fvs/fk evacuations
                    # aren't queued behind the (late-needed) fq evacuation.
                    def proj(nm, comp):
                        ps_c = psP.tile([128, T], f32, tag="p", name=f"psp_{t}_{h}_{nm}")
                        col0 = (h * 3 + comp) * 128
                        for k in range(KK):
                            nc.tensor.matmul(
                                ps_c,
                                lhsT=wq_sb[k][:, col0:col0 + 128],
                                rhs=xk[k],
                                start=(k == 0),
                                stop=(k == KK - 1),
                            )
                        csb = wpool.tile([128, T], f16, tag=f"c{h}_{nm}",
                                         name=f"{nm}_{t}_{h}")
                        nc.scalar.copy(csb, ps_c)
                        return csb
                    vs = proj("vs", 2)
                    fk_s = proj("fk", 1)
                    # v-path DFTs: fv (Gv), fvs (Gsm)
                    spec = {}
                    for nm, gcol in (("fv", 0), ("fvs", 128)):
                        ps_f = psS.tile([128, T], f32, tag="s", name=f"psf_{nm}_{t}_{h}")
                        nc.tensor.matmul(ps_f, lhsT=g_sb[:, gcol:gcol + 128], rhs=vs)
                        ssb = wpool.tile([128, T], f16, tag=f"{nm}{h}",
                                         name=f"{nm}_{t}_{h}")
                        nc.scalar.copy(ssb, ps_f)
                        spec[nm] = ssb
                    fv_s, fvs_s = spec["fv"], spec["fvs"]
                    fq_s = proj("fq", 0)
                    # lagged ifft: unit u-1's p12 gets an extra proj group of
                    # PE cover before its ifft matmuls hit the queue
                    if len(p12_pend) >= 1:
                        tt, hh, p12_o, vlist = p12_pend.pop(0)
                        emit_ifft(tt, hh, p12_o, vlist)
                    # fqs = half-swap of fq (rows j <-> 64+j) via GpSimd
                    # partition-shifted copies; rows 0/64 are polluted but
                    # multiplied by A2's zero rows downstream.
                    fqs = wpool.tile([128, T], f16, tag=f"fqs{h}", name=f"fqs_{t}_{h}")
                    nc.vector.tensor_copy(fqs[0:64, :], fq_s[64:128, :])
                    nc.vector.tensor_copy(fqs[64:128, :], fq_s[0:64, :])
                    # binding products written with partition-shifted outputs:
                    # in0 = [A ; D], in1 = [B ; -E] (no intermediate swap)
                    in0 = wpool.tile([128, T], f16, tag=f"in0_{h}", name=f"in0_{t}_{h}")
                    nc.vector.tensor_mul(in0[0:64, :], fk_s[0:64, :], fv_s[0:64, :])
                    nc.vector.tensor_mul(in0[64:128, :], fk_s[64:128, :], fvs_s[64:128, :])
                    in1 = wpool.tile([128, T], f16, tag=f"in1_{h}", name=f"in1_{t}_{h}")
                    nc.vector.tensor_mul(in1[0:64, :], fk_s[64:128, :], fv_s[64:128, :])
                    nc.vector.tensor_mul(in1[64:128, :], fk_s[0:64, :], fvs_s[0:64, :])
                    # causal cumsum: state = (in0 + state) - in1, carry-chained
                    kvt = kvpool.tile([128, T], f16, tag=f"kv{h}", name=f"kv_{t}_{h}")
                    init = 0.0 if t == 0 else kv_prev[h][:, T - 1:T]
                    nc.vector.tensor_tensor_scan(kvt, in0, in1, init, add, sub)
                    kv_prev[h] = kvt
                    # unbinding products
                    p12 = wpool.tile([128, 2 * T], f16, tag=f"p12{h}", name=f"p12_{t}_{h}")
                    nc.vector.tensor_mul(p12[:, 0:T], kvt, fq_s)
                    nc.vector.tensor_mul(p12[:, T:2 * T], kvt, fqs)
                    p12_pend.append((t, h, p12, vals_sb))
                # flush this chunk's remaining iffts (DVE evac so the Scalar
                # queue isn't the gate on the psV slot), then project out
                while p12_pend:
                    tt, hh, p12_o, vlist = p12_pend.pop(0)
                    emit_ifft(tt, hh, p12_o, vlist, dve_evac=True)
                # output projection (partial over this core's heads)
                last = t == NT - 1
                for od in range(D // 128):
                    ps_out = psV.tile([128, T], f32, tag="v", name=f"pso_{t}_{od}")
                    for h in range(HPC):
                        nc.tensor.matmul(ps_out,
                                         lhsT=wo_sb[h][:, od * 128:(od + 1) * 128],
                                         rhs=vals_sb[h],
                                         start=(h == 0), stop=(h == HPC - 1))
                    ot = wpool.tile([128, T], f16, tag="ot", name=f"ot_{t}_{od}")
                    if last:
                        # drain the kernel tail fast: half-size evacuations
                        # and DMAs so the final transfers start earlier and
                        # run on two DMA engines (128KB on one engine is
                        # ~10us -- it gates the end-of-kernel barrier)
                        eng = nc.vector.tensor_copy if od % 2 == 0 else nc.scalar.copy
                        o0, o1 = od * 128, (od + 1) * 128
                        eng(ot[:, 0:T // 2], ps_out[:, 0:T // 2])
                        nc.sync.dma_start(out=outT[o0:o1, t * T:t * T + T // 2],
                                          in_=ot[:, 0:T // 2])
                        eng(ot[:, T // 2:T], ps_out[:, T // 2:T])
                        nc.sync.dma_start(out=outT[o0:o1, t * T + T // 2:(t + 1) * T],
                                          in_=ot[:, T // 2:T])
                    else:
                        nc.vector.tensor_copy(ot, ps_out)
                        nc.sync.dma_start(out=outT[od * 128:(od + 1) * 128, ts], in_=ot)
    nc.compile()
    return nc


def _make_in_maps(x, w_qkv, w_out):
    gmat, amat, Gk16 = _build_consts()
    x16 = x.astype(np.float16)
    wqf = np.asarray(w_qkv, np.float64)
    wo16 = (w_out * (SV / SO)).astype(np.float16)
    in_maps = []
    for c in range(NCORES):
        b, g = divmod(c, 2)
        heads = range(4 * g, 4 * g + 4)
        blocks = []
        for h in heads:
            Wq_h = wqf[:, 0 * D + h * 128:0 * D + (h + 1) * 128]
            Wk_h = wqf[:, 1 * D + h * 128:1 * D + (h + 1) * 128]
            Wv_h = wqf[:, 2 * D + h * 128:2 * D + (h + 1) * 128]
            blocks.append((Wq_h @ Gk16).astype(np.float16))
            blocks.append((Wk_h @ Gk16).astype(np.float16))
            blocks.append(Wv_h.astype(np.float16))
        wq_cols = np.concatenate(blocks, axis=1)
        wo_rows = np.concatenate(
            [wo16[h * 128:(h + 1) * 128, :] for h in heads], axis=0)
        in_maps.append({
            "xT": np.ascontiguousarray(x16[b].T),
            "wq": np.ascontiguousarray(wq_cols),
            "wo": np.ascontiguousarray(wo_rows),
            "gmat": gmat,
            "amat": amat,
        })
    return in_maps


_NC_CACHE = None


def _get_program():
    global _NC_CACHE
    if _NC_CACHE is None:
        _NC_CACHE = _build_program()
    return _NC_CACHE


def kernel(x, w_qkv, w_out, _trace=False, _results_out=None):
    import sys
    if "/opt/trn_rl_repo" not in sys.path:
        sys.path.insert(0, "/opt/trn_rl_repo")
    from concourse.bass_utils import run_bass_kernel_spmd

    x = np.asarray(x)
    w_qkv = np.asarray(w_qkv)
    w_out = np.asarray(w_out)
    nc = _get_program()
    in_maps = _make_in_maps(x, w_qkv, w_out)
    res = run_bass_kernel_spmd(nc, in_maps, list(range(NCORES)), trace=_trace)
    if _results_out is not None:
        _results_out.append(res)
    out = np.empty((B, S, D), np.float32)
    for b in range(B):
        p0 = res.results[2 * b]["outT"].astype(np.float32)
        p1 = res.results[2 * b + 1]["outT"].astype(np.float32)
        out[b] = (p0 + p1).T * SO
    return out
